# revision 45
# baseline (speedup 1.0000x reference)
"""Trainium2 Bass kernel for nn_CoAttention.

Data parallel over batch: B=64 split as 8 batches on each of 8 NeuronCores.
Per batch item (Q = x[:384], D = x[384:]):
    qpT = tanh(W @ Q^T + b)              [768, 384]  (PE fp16, ACT tanh+bias)
    L   = Qp @ D^T natural layout        [384, 384]  via lhsT=qpT, rhs=dT
    E   = exp(L - SHIFT) fp32, r2 = rowsum(E)        (ACT exp, accum_out)
    P2  = E / r2        (row softmax, A_D)           fp16 + fp32r copies
    Ghat = E^T @ [P2 | 1]  -> r1 = last col          (fp32r PE matmul)
    GT  = Ghat[:, :384] / r1             [s, s']     (= (P2^T P1)^T, fp16)
    qp  = qpT^T via PE transposes
    Out1 = P2^T @ qp                     [384, 768]
    Out2 = GT^T(contract s) @ D          [384, 768]  (== P2^T @ (P1^T @ D))
    device stores [Out1 | Out2] fp16; the D passthrough third of the output
    is assembled on host directly from x.
Inputs are cast to fp16 on the host (halves HBM traffic; matmul rate on PE
is identical to fp32r). E stays fp32 on-chip (values up to ~e^32 overflow
fp16). A short stream of junk warm-up matmuls at kernel start trips the PE
HAM clock gate to 8/8 while the first tiles are still loading.
"""

import numpy as np
from contextlib import ExitStack

N_CORES = 8
BPC = 8          # batches per core
H = 768
T = 384
KT = H // 128    # 6
TT = T // 128    # 3
SHIFT = 60.0
N_WARMUP = 40

_cache = {}


def _build_nc():
    import concourse.bass as bass
    import concourse.tile as tile
    from concourse import bacc, mybir

    f32 = mybir.dt.float32
    f32r = mybir.dt.float32r
    f16 = mybir.dt.float16
    AF = mybir.ActivationFunctionType

    nc = bacc.Bacc()
    # inputs are host-packed to the exact SBUF tile layouts (partition-major)
    # so each DMA descriptor covers a full partition line (4.6-9.2KB)
    qt_h = nc.declare_dram_parameter("qtp", [BPC, 128, KT, T], f16, isOutput=False)
    dt_h = nc.declare_dram_parameter("dtp", [BPC, 128, KT, T], f16, isOutput=False)
    xd_h = nc.declare_dram_parameter("xdp", [BPC, 128, TT, H], f16, isOutput=False)
    wt_h = nc.declare_dram_parameter("wtp", [128, KT, H], f16, isOutput=False)
    b_h = nc.declare_dram_parameter("bias", [H], f32, isOutput=False)
    eye_h = nc.declare_dram_parameter("eye", [128, 128], f16, isOutput=False)
    out_h = nc.declare_dram_parameter("out", [BPC, T, 2 * H], f16, isOutput=True)

    with tile.TileContext(nc) as tc, ExitStack() as ctx:
        consts = ctx.enter_context(tc.tile_pool(name="consts", bufs=1))
        trp = ctx.enter_context(tc.tile_pool(name="trp", bufs=2))
        dp = ctx.enter_context(tc.tile_pool(name="dp", bufs=2))
        qpp = ctx.enter_context(tc.tile_pool(name="qpp", bufs=2))
        ep = ctx.enter_context(tc.tile_pool(name="ep", bufs=2))
        pp = ctx.enter_context(tc.tile_pool(name="pp", bufs=2))
        gp = ctx.enter_context(tc.tile_pool(name="gp", bufs=2))
        mop = ctx.enter_context(tc.tile_pool(name="mop", bufs=4))
        smallp = ctx.enter_context(tc.tile_pool(name="small", bufs=2))
        pps = ctx.enter_context(tc.tile_pool(name="pps", bufs=4, space="PSUM"))
        ppsT = ctx.enter_context(tc.tile_pool(name="ppsT", bufs=2, space="PSUM"))
        ppsG = ctx.enter_context(tc.tile_pool(name="ppsG", bufs=2, space="PSUM"))

        # Nothing moves until the ~7us engine/DMA preamble finishes. The
        # SWDGE (gpsimd, Q0) path measures ~5x the early throughput of the
        # HWDGE (sync, Q1) path, so ALL startup-critical bytes (wt + item0
        # qt) go first on gpsimd; everything else on sync.
        wt_sb = consts.tile([128, KT, H], f16)
        nc.gpsimd.dma_start(out=wt_sb, in_=wt_h[:, :, :])
        ident = consts.tile([128, 128], f16)
        nc.sync.dma_start(out=ident, in_=eye_h[:, :])
        bias_sb = consts.tile([128, KT], f32)
        nc.sync.dma_start(out=bias_sb, in_=b_h[:].rearrange("(oi p) -> p oi", p=128))
        negshift = consts.tile([128, 1], f32)
        nc.vector.memset(negshift, -SHIFT)
        junk = consts.tile([128, T], f16)
        nc.vector.memset(junk, 0.0)
        junklhs = consts.tile([128, 128], f16)
        nc.vector.memset(junklhs, 0.0)

        # warm-up: trip the HAM clock gate to K=8/8 and keep the PE busy
        # through the ~9us DMA pipeline cold-start. Entirely DMA-free
        # (memset operands) so it starts immediately.
        wps = pps.tile([128, T], f32, tag="ps")
        for _ in range(N_WARMUP):
            nc.tensor.matmul(wps, junklhs, junk, start=True, stop=True)

        cp_i = 0

        def copy_out(dst, src):
            # rotate PSUM->SBUF copies 2:1 between vector and scalar (gpsimd
            # cannot access PSUM)
            nonlocal cp_i
            if cp_i % 3 < 2:
                nc.vector.tensor_copy(dst, src)
            else:
                nc.scalar.activation(dst, src, AF.Copy)
            cp_i += 1

        st = {}  # per-item tile state for the 1-item software pipeline skew

        def front(b):
            """loads + step1 + L/softmax + qp transposes for item b."""
            # ---- loads (qt first: it gates step1; dt/d16 needed later) ----
            qt = trp.tile([128, KT, T], f16, tag="qt")
            nc.gpsimd.dma_start(out=qt, in_=qt_h[b])
            dt = trp.tile([128, KT, T], f16, tag="dt")
            nc.sync.dma_start(out=dt, in_=dt_h[b])
            d16 = dp.tile([128, TT, H], f16, tag="d16")
            d16_eng = nc.sync if b == 0 else nc.gpsimd
            d16_eng.dma_start(out=d16, in_=xd_h[b])

            # ---- step1: qpT = tanh(W @ Q^T + b), [h_out, t] fp16 ----
            qpT = qpp.tile([128, KT, T], f16, tag="qpT")
            for oi in range(KT):
                ps = pps.tile([128, T], f32, tag="ps")
                for ki in range(KT):
                    nc.tensor.matmul(ps, wt_sb[:, ki, oi * 128:(oi + 1) * 128],
                                     qt[:, ki, :],
                                     start=(ki == 0), stop=(ki == KT - 1))
                nc.scalar.activation(qpT[:, oi, :], ps, AF.Tanh, bias=bias_sb[:, oi:oi + 1])

            # ---- L natural [t, s]; E = exp(L-SHIFT) fp32; r2 = rowsum;
            # ---- P2 = E/r2 in f32r (ACT copy-with-scale) + fp16 cast ----
            e_sb = ep.tile([128, TT, T], f32r, tag="E")
            r2 = smallp.tile([128, TT], f32, tag="r2")
            r2r = smallp.tile([128, TT], f32, tag="r2r")
            p2f = pp.tile([128, TT, T], f32r, tag="p2f")
            p2h = pp.tile([128, TT, T], f16, tag="p2h")
            for ti in range(TT):
                ps = pps.tile([128, T], f32, tag="ps")
                for ki in range(KT):
                    nc.tensor.matmul(ps, qpT[:, ki, ti * 128:(ti + 1) * 128],
                                     dt[:, ki, :],
                                     start=(ki == 0), stop=(ki == KT - 1))
                nc.scalar.activation(e_sb[:, ti, :], ps, AF.Exp, bias=negshift[:, 0:1],
                                     accum_out=r2[:, ti:ti + 1])
                nc.vector.reciprocal(r2r[:, ti:ti + 1], r2[:, ti:ti + 1])
                nc.scalar.activation(p2f[:, ti, :], e_sb[:, ti, :], AF.Copy,
                                     scale=r2r[:, ti:ti + 1])
                nc.vector.tensor_copy(p2h[:, ti, :], p2f[:, ti, :])

            # ---- qp natural [t, h] fp16 via PE transposes ----
            qp = qpp.tile([128, TT, H], f16, tag="qp")
            for ti in range(TT):
                for hf in range(2):
                    ps = ppsT.tile([128, T], f16, tag="psT")
                    for j in range(TT):
                        nc.tensor.transpose(ps[:, j * 128:(j + 1) * 128],
                                            qpT[:, hf * TT + j, ti * 128:(ti + 1) * 128],
                                            ident)
                    nc.vector.tensor_copy(qp[:, ti, hf * T:(hf + 1) * T], ps)
            st[b] = (e_sb, p2f, p2h, qp, d16)

        def back(b):
            """Ghat + Out1 + Out2 + stores for item b (runs after front(b+1)
            on the PE, so the softmax chain latency is fully hidden)."""
            e_sb, p2f, p2h, qp, d16 = st.pop(b)
            ob = out_h[b].rearrange("(si p) c -> p si c", p=128)

            # ---- Ghat = E^T @ P2; r1 = rowsum(Ghat) (P2 rows sum to 1);
            # ---- GT = Ghat/r1 fp16 ----
            gT = gp.tile([128, TT, T], f16, tag="gT")
            gf = gp.tile([128, TT, T], f32, tag="gf")
            r1 = smallp.tile([128, TT], f32, tag="r1")
            r1r = smallp.tile([128, TT], f32, tag="r1r")
            for si in range(TT):
                ps = ppsG.tile([128, T], f32, tag="psG")
                for ti in range(TT):
                    nc.tensor.matmul(ps, e_sb[:, ti, si * 128:(si + 1) * 128],
                                     p2f[:, ti, :],
                                     start=(ti == 0), stop=(ti == TT - 1))
                nc.scalar.activation(gf[:, si, :], ps, AF.Identity,
                                     accum_out=r1[:, si:si + 1])
                nc.vector.reciprocal(r1r[:, si:si + 1], r1[:, si:si + 1])
                nc.scalar.activation(gT[:, si, :], gf[:, si, :], AF.Copy,
                                     scale=r1r[:, si:si + 1])

            # ---- Out1 = P2^T @ qp (both h-halves share each lhsT load) ----
            o1 = mop.tile([128, TT, H], f16, tag="mo")
            for sp in range(TT):
                psA = pps.tile([128, T], f32, tag="ps")
                psB = pps.tile([128, T], f32, tag="ps")
                for ti in range(TT):
                    lhs = p2h[:, ti, sp * 128:(sp + 1) * 128]
                    nc.tensor.matmul(psA, lhs, qp[:, ti, 0:T],
                                     start=(ti == 0), stop=(ti == TT - 1))
                    nc.tensor.matmul(psB, lhs, qp[:, ti, T:2 * T],
                                     start=(ti == 0), stop=(ti == TT - 1))
                copy_out(o1[:, sp, 0:T], psA)
                copy_out(o1[:, sp, T:2 * T], psB)

            # ---- Out2 = GT^T @ D (contract s) ----
            o2 = mop.tile([128, TT, H], f16, tag="mo")
            for sp in range(TT):
                psA = pps.tile([128, T], f32, tag="ps")
                psB = pps.tile([128, T], f32, tag="ps")
                for si in range(TT):
                    lhs = gT[:, si, sp * 128:(sp + 1) * 128]
                    nc.tensor.matmul(psA, lhs, d16[:, si, 0:T],
                                     start=(si == 0), stop=(si == TT - 1))
                    nc.tensor.matmul(psB, lhs, d16[:, si, T:2 * T],
                                     start=(si == 0), stop=(si == TT - 1))
                copy_out(o2[:, sp, 0:T], psA)
                copy_out(o2[:, sp, T:2 * T], psB)

            o2_eng = nc.gpsimd if b == BPC - 1 else nc.sync
            nc.sync.dma_start(out=ob[:, :, 0:H], in_=o1)
            o2_eng.dma_start(out=ob[:, :, H:2 * H], in_=o2)

        for b in range(BPC):
            front(b)
            if b >= 1:
                back(b - 1)
        back(BPC - 1)

    nc.compile()
    return nc


def get_nc():
    if "nc" not in _cache:
        _cache["nc"] = _build_nc()
    return _cache["nc"]


def _prep(x, W, b):
    B = x.shape[0]
    x = np.asarray(x, dtype=np.float32)
    xt = np.swapaxes(x, 1, 2).astype(np.float16)          # [B, h, t']
    # pack to SBUF tile layouts: [B, 128(p), ki/si, free]
    qtp = np.ascontiguousarray(
        xt[:, :, 0:T].reshape(B, KT, 128, T).transpose(0, 2, 1, 3))
    dtp = np.ascontiguousarray(
        xt[:, :, T:H].reshape(B, KT, 128, T).transpose(0, 2, 1, 3))
    xdp = np.ascontiguousarray(
        x[:, T:, :].astype(np.float16).reshape(B, TT, 128, H).transpose(0, 2, 1, 3))
    WT = np.asarray(W, dtype=np.float32).T.astype(np.float16)
    wtp = np.ascontiguousarray(WT.reshape(KT, 128, H).transpose(1, 0, 2))
    bias = np.ascontiguousarray(np.asarray(b, dtype=np.float32))
    eye = np.eye(128, dtype=np.float16)
    in_maps = [{"qtp": qtp[i * BPC:(i + 1) * BPC], "dtp": dtp[i * BPC:(i + 1) * BPC],
                "xdp": xdp[i * BPC:(i + 1) * BPC], "wtp": wtp, "bias": bias, "eye": eye}
               for i in range(N_CORES)]
    return in_maps


def run(x, W, b, trace=False, tmpdir=None):
    from concourse.bass_utils import run_bass_kernel_spmd
    nc = get_nc()
    x = np.asarray(x, dtype=np.float32)
    res = run_bass_kernel_spmd(nc, _prep(x, W, b), list(range(N_CORES)),
                               trace=trace, tmpdir=tmpdir)
    dev = np.concatenate([res.results[i]["out"] for i in range(N_CORES)], axis=0)
    out = np.empty((x.shape[0], T, 3 * H), dtype=np.float32)
    out[:, :, 0:2 * H] = dev.astype(np.float32)
    out[:, :, 2 * H:] = x[:, T:, :]
    return out, res


def kernel(x, W, b):
    return run(x, W, b)[0]


# revision 46
# speedup vs baseline: 1.1437x; 1.1437x over previous
"""Trainium2 Bass kernel for nn_CoAttention.

Data parallel over batch: B=64 split as 8 batches on each of 8 NeuronCores.
Per batch item (Q = x[:384], D = x[384:]):
    qpT = tanh(W @ Q^T + b)              [768, 384]  (PE fp16, ACT tanh+bias)
    L   = Qp @ D^T natural layout        [384, 384]  via lhsT=qpT, rhs=dT
    E   = exp(L - SHIFT) fp32, r2 = rowsum(E)        (ACT exp, accum_out)
    P2  = E / r2        (row softmax, A_D)           fp16 + fp32r copies
    Ghat = E^T @ [P2 | 1]  -> r1 = last col          (fp32r PE matmul)
    GT  = Ghat[:, :384] / r1             [s, s']     (= (P2^T P1)^T, fp16)
    qp  = qpT^T via PE transposes
    Out1 = P2^T @ qp                     [384, 768]
    Out2 = GT^T(contract s) @ D          [384, 768]  (== P2^T @ (P1^T @ D))
    device stores [Out1 | Out2] fp16; the D passthrough third of the output
    is assembled on host directly from x.
Inputs are cast to fp16 on the host (halves HBM traffic; matmul rate on PE
is identical to fp32r). E stays fp32 on-chip (values up to ~e^32 overflow
fp16). A short stream of junk warm-up matmuls at kernel start trips the PE
HAM clock gate to 8/8 while the first tiles are still loading.
"""

import numpy as np
from contextlib import ExitStack

N_CORES = 8
BPC = 8          # batches per core
H = 768
T = 384
KT = H // 128    # 6
TT = T // 128    # 3
SHIFT = 60.0
N_WARMUP = 40

_cache = {}


def _build_nc():
    import concourse.bass as bass
    import concourse.tile as tile
    from concourse import bacc, mybir

    f32 = mybir.dt.float32
    f32r = mybir.dt.float32r
    f16 = mybir.dt.float16
    AF = mybir.ActivationFunctionType

    nc = bacc.Bacc()
    # inputs are host-packed to the exact SBUF tile layouts (partition-major)
    # so each DMA descriptor covers a full partition line (4.6-9.2KB)
    qt_h = nc.declare_dram_parameter("qtp", [BPC, 128, KT, T], f16, isOutput=False)
    dt_h = nc.declare_dram_parameter("dtp", [BPC, 128, KT, T], f16, isOutput=False)
    xd_h = nc.declare_dram_parameter("xdp", [BPC, 128, TT, H], f16, isOutput=False)
    wt_h = nc.declare_dram_parameter("wtp", [128, KT, H], f16, isOutput=False)
    b_h = nc.declare_dram_parameter("bias", [H], f32, isOutput=False)
    eye_h = nc.declare_dram_parameter("eye", [128, 128], f16, isOutput=False)
    out_h = nc.declare_dram_parameter("out", [BPC, T, 2 * H], f16, isOutput=True)

    with tile.TileContext(nc) as tc, ExitStack() as ctx:
        consts = ctx.enter_context(tc.tile_pool(name="consts", bufs=1))
        trp = ctx.enter_context(tc.tile_pool(name="trp", bufs=2))
        dp = ctx.enter_context(tc.tile_pool(name="dp", bufs=2))
        qpp = ctx.enter_context(tc.tile_pool(name="qpp", bufs=2))
        ep = ctx.enter_context(tc.tile_pool(name="ep", bufs=2))
        pp = ctx.enter_context(tc.tile_pool(name="pp", bufs=2))
        gp = ctx.enter_context(tc.tile_pool(name="gp", bufs=2))
        mop = ctx.enter_context(tc.tile_pool(name="mop", bufs=4))
        smallp = ctx.enter_context(tc.tile_pool(name="small", bufs=2))
        pps = ctx.enter_context(tc.tile_pool(name="pps", bufs=4, space="PSUM"))
        ppsT = ctx.enter_context(tc.tile_pool(name="ppsT", bufs=2, space="PSUM"))
        ppsG = ctx.enter_context(tc.tile_pool(name="ppsG", bufs=2, space="PSUM"))

        # Nothing moves until the ~7us engine/DMA preamble finishes. The
        # SWDGE (gpsimd, Q0) path measures ~5x the early throughput of the
        # HWDGE (sync, Q1) path, so ALL startup-critical bytes (wt + item0
        # qt) go first on gpsimd; everything else on sync.
        wt_sb = consts.tile([128, KT, H], f16)
        nc.gpsimd.dma_start(out=wt_sb, in_=wt_h[:, :, :])
        ident = consts.tile([128, 128], f16)
        nc.sync.dma_start(out=ident, in_=eye_h[:, :])
        bias_sb = consts.tile([128, KT], f32)
        nc.sync.dma_start(out=bias_sb, in_=b_h[:].rearrange("(oi p) -> p oi", p=128))
        negshift = consts.tile([128, 1], f32)
        nc.vector.memset(negshift, -SHIFT)
        junk = consts.tile([128, T], f16)
        nc.vector.memset(junk, 0.0)
        junklhs = consts.tile([128, 128], f16)
        nc.vector.memset(junklhs, 0.0)

        # warm-up: trip the HAM clock gate to K=8/8 and keep the PE busy
        # through the ~9us DMA pipeline cold-start. Entirely DMA-free
        # (memset operands) so it starts immediately.
        wps = pps.tile([128, T], f32, tag="ps")
        for _ in range(N_WARMUP):
            nc.tensor.matmul(wps, junklhs, junk, start=True, stop=True)

        cp_i = 0

        def copy_out(dst, src):
            # rotate PSUM->SBUF copies 2:1 between vector and scalar (gpsimd
            # cannot access PSUM)
            nonlocal cp_i
            if cp_i % 3 < 2:
                nc.vector.tensor_copy(dst, src)
            else:
                nc.scalar.activation(dst, src, AF.Copy)
            cp_i += 1

        st = {}  # per-item tile state for the 1-item software pipeline skew

        def front(b):
            """loads + step1 + L/softmax + qp transposes for item b."""
            # ---- loads (qt first: it gates step1; dt/d16 needed later) ----
            qt = trp.tile([128, KT, T], f16, tag="qt")
            nc.gpsimd.dma_start(out=qt, in_=qt_h[b])
            dt = trp.tile([128, KT, T], f16, tag="dt")
            nc.sync.dma_start(out=dt, in_=dt_h[b])
            d16 = dp.tile([128, TT, H], f16, tag="d16")
            d16_eng = nc.sync if b == 0 else nc.gpsimd
            d16_eng.dma_start(out=d16, in_=xd_h[b])

            # ---- step1: qpT = tanh(W @ Q^T + b), [h_out, t] fp16 ----
            qpT = qpp.tile([128, KT, T], f16, tag="qpT")
            for oi in range(KT):
                ps = pps.tile([128, T], f32, tag="ps")
                for ki in range(KT):
                    nc.tensor.matmul(ps, wt_sb[:, ki, oi * 128:(oi + 1) * 128],
                                     qt[:, ki, :],
                                     start=(ki == 0), stop=(ki == KT - 1))
                nc.scalar.activation(qpT[:, oi, :], ps, AF.Tanh, bias=bias_sb[:, oi:oi + 1])

            # ---- L natural [t, s]; E = exp(L-SHIFT) fp32; r2 = rowsum;
            # ---- P2 = E/r2 in f32r (ACT copy-with-scale) + fp16 cast ----
            e_sb = ep.tile([128, TT, T], f32r, tag="E")
            r2 = smallp.tile([128, TT], f32, tag="r2")
            r2r = smallp.tile([128, TT], f32, tag="r2r")
            p2f = pp.tile([128, TT, T], f32r, tag="p2f")
            p2h = pp.tile([128, TT, T], f16, tag="p2h")
            for ti in range(TT):
                ps = pps.tile([128, T], f32, tag="ps")
                for ki in range(KT):
                    nc.tensor.matmul(ps, qpT[:, ki, ti * 128:(ti + 1) * 128],
                                     dt[:, ki, :],
                                     start=(ki == 0), stop=(ki == KT - 1))
                nc.scalar.activation(e_sb[:, ti, :], ps, AF.Exp, bias=negshift[:, 0:1],
                                     accum_out=r2[:, ti:ti + 1])
                nc.vector.reciprocal(r2r[:, ti:ti + 1], r2[:, ti:ti + 1])
                nc.scalar.activation(p2f[:, ti, :], e_sb[:, ti, :], AF.Copy,
                                     scale=r2r[:, ti:ti + 1])
                nc.vector.tensor_copy(p2h[:, ti, :], p2f[:, ti, :])

            # ---- qp natural [t, h] fp16 via PE transposes ----
            qp = qpp.tile([128, TT, H], f16, tag="qp")
            for ti in range(TT):
                for hf in range(2):
                    ps = ppsT.tile([128, T], f16, tag="psT")
                    for j in range(TT):
                        nc.tensor.transpose(ps[:, j * 128:(j + 1) * 128],
                                            qpT[:, hf * TT + j, ti * 128:(ti + 1) * 128],
                                            ident)
                    nc.vector.tensor_copy(qp[:, ti, hf * T:(hf + 1) * T], ps)
            st[b] = (e_sb, p2f, p2h, qp, d16)

        def back(b):
            """Ghat + Out1 + Out2 + stores for item b (runs after front(b+1)
            on the PE, so the softmax chain latency is fully hidden)."""
            e_sb, p2f, p2h, qp, d16 = st.pop(b)
            ob = out_h[b].rearrange("(si p) c -> p si c", p=128)

            # ---- Ghat = E^T @ P2; r1 = rowsum(Ghat) (P2 rows sum to 1);
            # ---- GT = Ghat/r1 fp16 ----
            gT = gp.tile([128, TT, T], f16, tag="gT")
            gf = gp.tile([128, TT, T], f32, tag="gf")
            r1 = smallp.tile([128, TT], f32, tag="r1")
            r1r = smallp.tile([128, TT], f32, tag="r1r")
            for si in range(TT):
                ps = ppsG.tile([128, T], f32, tag="psG")
                for ti in range(TT):
                    nc.tensor.matmul(ps, e_sb[:, ti, si * 128:(si + 1) * 128],
                                     p2f[:, ti, :],
                                     start=(ti == 0), stop=(ti == TT - 1))
                nc.scalar.activation(gf[:, si, :], ps, AF.Identity,
                                     accum_out=r1[:, si:si + 1])
                nc.vector.reciprocal(r1r[:, si:si + 1], r1[:, si:si + 1])
                nc.scalar.activation(gT[:, si, :], gf[:, si, :], AF.Copy,
                                     scale=r1r[:, si:si + 1])

            # ---- Out1 = P2^T @ qp (both h-halves share each lhsT load) ----
            o1 = mop.tile([128, TT, H], f16, tag="mo")
            for sp in range(TT):
                psA = pps.tile([128, T], f32, tag="ps")
                psB = pps.tile([128, T], f32, tag="ps")
                for ti in range(TT):
                    lhs = p2h[:, ti, sp * 128:(sp + 1) * 128]
                    nc.tensor.matmul(psA, lhs, qp[:, ti, 0:T],
                                     start=(ti == 0), stop=(ti == TT - 1))
                    nc.tensor.matmul(psB, lhs, qp[:, ti, T:2 * T],
                                     start=(ti == 0), stop=(ti == TT - 1))
                copy_out(o1[:, sp, 0:T], psA)
                copy_out(o1[:, sp, T:2 * T], psB)

            # ---- Out2 = GT^T @ D (contract s) ----
            o2 = mop.tile([128, TT, H], f16, tag="mo")
            for sp in range(TT):
                psA = pps.tile([128, T], f32, tag="ps")
                psB = pps.tile([128, T], f32, tag="ps")
                for si in range(TT):
                    lhs = gT[:, si, sp * 128:(sp + 1) * 128]
                    nc.tensor.matmul(psA, lhs, d16[:, si, 0:T],
                                     start=(si == 0), stop=(si == TT - 1))
                    nc.tensor.matmul(psB, lhs, d16[:, si, T:2 * T],
                                     start=(si == 0), stop=(si == TT - 1))
                copy_out(o2[:, sp, 0:T], psA)
                copy_out(o2[:, sp, T:2 * T], psB)

            o2_eng = nc.gpsimd if b == BPC - 1 else nc.sync
            nc.sync.dma_start(out=ob[:, :, 0:H], in_=o1)
            o2_eng.dma_start(out=ob[:, :, H:2 * H], in_=o2)

        for b in range(BPC):
            front(b)
            back(b)

    nc.compile()
    return nc


def get_nc():
    if "nc" not in _cache:
        _cache["nc"] = _build_nc()
    return _cache["nc"]


def _prep(x, W, b):
    B = x.shape[0]
    x = np.asarray(x, dtype=np.float32)
    xt = np.swapaxes(x, 1, 2).astype(np.float16)          # [B, h, t']
    # pack to SBUF tile layouts: [B, 128(p), ki/si, free]
    qtp = np.ascontiguousarray(
        xt[:, :, 0:T].reshape(B, KT, 128, T).transpose(0, 2, 1, 3))
    dtp = np.ascontiguousarray(
        xt[:, :, T:H].reshape(B, KT, 128, T).transpose(0, 2, 1, 3))
    xdp = np.ascontiguousarray(
        x[:, T:, :].astype(np.float16).reshape(B, TT, 128, H).transpose(0, 2, 1, 3))
    WT = np.asarray(W, dtype=np.float32).T.astype(np.float16)
    wtp = np.ascontiguousarray(WT.reshape(KT, 128, H).transpose(1, 0, 2))
    bias = np.ascontiguousarray(np.asarray(b, dtype=np.float32))
    eye = np.eye(128, dtype=np.float16)
    in_maps = [{"qtp": qtp[i * BPC:(i + 1) * BPC], "dtp": dtp[i * BPC:(i + 1) * BPC],
                "xdp": xdp[i * BPC:(i + 1) * BPC], "wtp": wtp, "bias": bias, "eye": eye}
               for i in range(N_CORES)]
    return in_maps


def run(x, W, b, trace=False, tmpdir=None):
    from concourse.bass_utils import run_bass_kernel_spmd
    nc = get_nc()
    x = np.asarray(x, dtype=np.float32)
    res = run_bass_kernel_spmd(nc, _prep(x, W, b), list(range(N_CORES)),
                               trace=trace, tmpdir=tmpdir)
    dev = np.concatenate([res.results[i]["out"] for i in range(N_CORES)], axis=0)
    out = np.empty((x.shape[0], T, 3 * H), dtype=np.float32)
    out[:, :, 0:2 * H] = dev.astype(np.float32)
    out[:, :, 2 * H:] = x[:, T:, :]
    return out, res


def kernel(x, W, b):
    return run(x, W, b)[0]
